# revision 1
# baseline (speedup 1.0000x reference)
"""Causal attention kernel for Trainium2 (Bass/Tile), 8-core SPMD.

Problem: B=16, S=2048, D=128 fp32 causal attention
    scores = Q @ K^T            (per batch)
    scores -= INF * triu(k=1)   (before scaling, as in reference)
    attn = softmax(scores / sqrt(D))
    out = attn @ V

Sharding: batch dim across 8 cores, 2 batches per core, no communication.

Per-core dataflow (per batch, per 512-wide q-block, per 128-wide k-chunk):
    Q^T/K^T are pre-transposed on the host and DMA'd straight in.
    diag chunks: causal mask written by a PE const matmul (u01.T @ -1e9*I,
                 start=True), scores accumulate on top with start=False
    S^T[k, q] = (K^T chunk).T @ Q^T slice      (contract d on partitions)
    P^T = exp(S^T * 1/sqrt(D))                 (ACT, PSUM -> SBUF fp32r)
    O^T[d, q] += V_chunk.T @ P^T chunk          (PSUM accumulate)
    l[q]      += allones.T @ P^T chunk          (rowsum, broadcast on all rows)
    out stays [d, q]: O^T * (1/l) on DVE, DMA'd out; the host gather
    transposes back to [q, d].

All matmuls run in fp32r (full-rate fp32 on the PE, ~213ns/512-col warm).
Software pipelining: PV/rowsum of group g is emitted after S+exp of group
g+1, and each q-block's evacuation is deferred past the next block's first
group, so the in-order PE queue never stalls on ACT or DVE chains.
"""

import os

os.environ.setdefault("MYCRO_LOCAL_CACHE", "1")

import math

import numpy as np

import concourse.bass as bass
import concourse.mybir as mybir
import concourse.tile as tile
from concourse import bacc
from concourse.bass_utils import run_bass_kernel_spmd
from concourse.masks import make_identity

F32 = mybir.dt.float32
F32R = mybir.dt.float32r
EXPF = mybir.ActivationFunctionType.Exp

N_CORES = 8
B = 16
S = 2048
D = 128
BPC = B // N_CORES  # batches per core
SCALE = 1.0 / math.sqrt(float(D))
NEG = -1.0e9
NQB = S // 512  # q blocks per batch
NCH = S // 128  # k chunks per batch


def build():
    nc = bacc.Bacc("TRN2", target_bir_lowering=False, debug=False, num_devices=N_CORES)
    q_d = nc.dram_tensor("qt", [BPC, D, S], F32, kind="ExternalInput")
    k_d = nc.dram_tensor("kt", [BPC, D, S], F32, kind="ExternalInput")
    v_d = nc.dram_tensor("v", [BPC, S, D], F32, kind="ExternalInput")
    o_d = nc.dram_tensor("o", [BPC, D, S], F32, kind="ExternalOutput")
    l_d = nc.dram_tensor("l", [BPC, NQB, 1, 512], F32, kind="ExternalOutput")

    with tile.TileContext(nc) as tc:
        with (
            tc.tile_pool(name="const", bufs=1) as constp,
            tc.tile_pool(name="tpose", bufs=2) as tposep,
            tc.tile_pool(name="pt", bufs=6) as ptp,
            tc.tile_pool(name="evac", bufs=2) as evacp,
            tc.tile_pool(name="stps", bufs=3, space="PSUM") as stps,
            tc.tile_pool(name="otps", bufs=1, space="PSUM") as otps,
            tc.tile_pool(name="lps", bufs=1, space="PSUM") as lps,
        ):
            # ---- constants (mask consts first: qb0 g0 needs them) ----
            ident32 = constp.tile([128, 128], F32, name="ident32")
            make_identity(nc, ident32[:])
            zb = constp.tile([128, 1], F32, name="zb")
            nc.gpsimd.memset(zb[:], 0.0)
            # dummy exp: ACT table load starts immediately
            warm_exp = constp.tile([128, 1], F32, name="warm_exp")
            nc.scalar.activation(
                warm_exp[:], zb[:], EXPF, bias=zb[:], scale=SCALE
            )
            # u01[i, k] = 1 iff i < k; with rhs=-1e9*I the product writes the
            # causal mask NEG*[q < k] straight into PSUM on the PE.
            u01_f = constp.tile([128, 128], F32, name="u01_f")
            nc.gpsimd.memset(u01_f[:], 1.0)
            nc.gpsimd.affine_select(
                out=u01_f[:],
                in_=u01_f[:],
                compare_op=mybir.AluOpType.is_ge,
                fill=0.0,
                base=-1,
                pattern=[[1, 128]],
                channel_multiplier=-1,
            )
            u01_r = constp.tile([128, 128], F32R, name="u01_r")
            nc.vector.tensor_copy(u01_r[:], u01_f[:])
            idneg_r = constp.tile([128, 128], F32R, name="idneg_r")
            with nc.allow_low_precision("f32r is full-width fp32 storage"):
                nc.vector.tensor_scalar_mul(idneg_r[:], ident32[:], NEG)
            idneg2_r = constp.tile([128, 384], F32R, name="idneg2_r")
            nc.gpsimd.memset(idneg2_r[:].bitcast(F32), 0.0)
            with nc.allow_low_precision("f32r is full-width fp32 storage"):
                nc.vector.tensor_scalar_mul(
                    idneg2_r[:, 0:128], ident32[:], NEG
                )
                nc.vector.tensor_scalar_mul(
                    idneg2_r[:, 256:384], ident32[:], NEG
                )
            ones_f = constp.tile([128, 128], F32, name="ones_f")
            nc.gpsimd.memset(ones_f[:], 1.0)
            ones_r = constp.tile([128, 128], F32R, name="ones_r")
            nc.vector.tensor_copy(ones_r[:], ones_f[:])

            # HAM warmup: dense PE activity while the first DMAs land
            warm_ps = stps.tile([128, 128], F32, name="warm_ps", tag="stps")
            for _ in range(30):
                nc.tensor.matmul(
                    warm_ps[:], u01_r[:], u01_r[:], start=True, stop=True
                )

            # software-pipeline state: PV/rowsum of group g is emitted
            # after S+exp of group g+1 (PE never queues behind exp); the
            # transpose/store tail of q-block N is emitted during N+1.
            pending_pv = [None]
            pending_evac = [None]

            def flush_pv():
                if pending_pv[0] is not None:
                    pending_pv[0]()
                    pending_pv[0] = None

            def flush_evac():
                if pending_evac[0] is not None:
                    pending_evac[0]()
                    pending_evac[0] = None

            for b in range(BPC):
                # ---- load Q^T, K^T (host pre-transposed), V via cast DMA ----
                qt = tposep.tile([128, S], F32R, name="qt")
                kt = tposep.tile([128, S], F32R, name="kt")
                vr = tposep.tile([128, S], F32R, name="vr")
                nc.sync.dma_start(qt[:, 0:1024], q_d[b, :, 0:1024].bitcast(F32R))
                nc.sync.dma_start(kt[:, 0:1024], k_d[b, :, 0:1024].bitcast(F32R))
                # vr[:, j*128 + d] = V[b, j*128 + p, d]
                nc.sync.dma_start(
                    vr[:, 0:1024].rearrange("p (j d) -> p j d", d=128),
                    v_d[b, 0:1024].rearrange("(j p) d -> p j d", p=128).bitcast(F32R),
                )
                nc.sync.dma_start(qt[:, 1024:2048], q_d[b, :, 1024:2048].bitcast(F32R))
                nc.sync.dma_start(kt[:, 1024:2048], k_d[b, :, 1024:2048].bitcast(F32R))
                nc.sync.dma_start(
                    vr[:, 1024:2048].rearrange("p (j d) -> p j d", d=128),
                    v_d[b, 1024:2048]
                    .rearrange("(j p) d -> p j d", p=128)
                    .bitcast(F32R),
                )

                # ---- q blocks ----
                for qb in range(NQB):
                    n_full = 4 * qb
                    n_ch = n_full + 4
                    q0 = qb * 512

                    # (chunks, extent, is_diag); st tiles are [128, 1024]
                    groups = []
                    jf = 0
                    while jf < n_full:
                        g = min(2, n_full - jf)
                        groups.append(
                            (
                                [(jf + c, 0, 512, c * 512) for c in range(g)],
                                g * 512,
                                False,
                            )
                        )
                        jf += g
                    groups.append(
                        ([(n_full, 0, 512, 0), (n_full + 1, 128, 384, 512)], 896, True)
                    )
                    groups.append(
                        (
                            [(n_full + 2, 256, 256, 0), (n_full + 3, 384, 128, 256)],
                            384,
                            True,
                        )
                    )

                    ot = otps.tile([128, 512], F32, name="ot")
                    lp = lps.tile([128, 512], F32, name="lp", tag="lp")

                    for gi, (chunks, extent, is_diag) in enumerate(groups):
                        st = stps.tile([128, 1024], F32, name="st", tag="stps")
                        if is_diag and chunks[0][3] == 0 and len(chunks) == 2 and chunks[1][3] == 256:
                            # diag B: one mask matmul covers both chunks
                            nc.tensor.matmul(
                                st[:, 0:384],
                                u01_r[:],
                                idneg2_r[:],
                                start=True,
                                stop=False,
                            )
                            premasked = True
                        else:
                            premasked = False
                        for (j, qoff, width, col) in chunks:
                            if is_diag and not premasked:
                                # write NEG*[q<k] into the first 128 cols,
                                # then accumulate the scores on top
                                nc.tensor.matmul(
                                    st[:, col : col + 128],
                                    u01_r[:],
                                    idneg_r[:],
                                    start=True,
                                    stop=False,
                                )
                            nc.tensor.matmul(
                                st[:, col : col + width],
                                kt[:, j * 128 : (j + 1) * 128],
                                qt[:, q0 + qoff : q0 + qoff + width],
                                start=not is_diag,
                                stop=True,
                            )
                        pt = ptp.tile([128, 1024], F32R, name="pt", tag="pt")
                        nc.scalar.activation(
                            pt[:, 0:extent],
                            st[:, 0:extent],
                            EXPF,
                            bias=zb[:],
                            scale=SCALE,
                        )
                        flush_pv()
                        if gi == 1:
                            flush_evac()

                        def pv(
                            chunks=chunks,
                            ot=ot,
                            lp=lp,
                            pt=pt,
                            vr=vr,
                            n_ch=n_ch,
                            is_last=(gi == len(groups) - 1),
                            b=b,
                            q0=q0,
                        ):
                            for (j, qoff, width, col) in chunks:
                                nc.tensor.matmul(
                                    ot[:, qoff : qoff + width],
                                    vr[:, j * 128 : (j + 1) * 128],
                                    pt[:, col : col + width],
                                    start=(j == 0),
                                    stop=(j == n_ch - 1),
                                )
                                nc.tensor.matmul(
                                    lp[:, qoff : qoff + width],
                                    ones_r[:],
                                    pt[:, col : col + width],
                                    start=(j == 0),
                                    stop=(j == n_ch - 1),
                                )
                            if not is_last:
                                return
                            # ---- evacuation: ship unnormalized O^T and
                            # the rowsum row; the host divides (fp64).
                            # lp rows are all equal (all-ones stationary).
                            ots = evacp.tile([128, 512], F32, name="ots")
                            nc.vector.tensor_copy(ots[:], ot[:])
                            lsb = evacp.tile([1, 512], F32, name="lsb")
                            nc.vector.tensor_copy(lsb[:], lp[0:1, :])

                            def evac(b=b, q0=q0, ots=ots, lsb=lsb):
                                nc.sync.dma_start(
                                    o_d[b, :, q0 : q0 + 512], ots[:]
                                )
                                nc.sync.dma_start(
                                    l_d[b, q0 // 512], lsb[:]
                                )

                            pending_evac[0] = evac

                        pending_pv[0] = pv

            flush_pv()
            flush_evac()
    nc.compile()
    return nc


_NC_CACHE = None


def _get_nc():
    global _NC_CACHE
    if _NC_CACHE is None:
        _NC_CACHE = build()
    return _NC_CACHE


def kernel(query, key, value, _trace=False):
    nc = _get_nc()
    in_maps = []
    for c in range(N_CORES):
        sl = slice(c * BPC, (c + 1) * BPC)
        in_maps.append(
            {
                "qt": np.ascontiguousarray(
                    np.asarray(query[sl], dtype=np.float32).transpose(0, 2, 1)
                ),
                "kt": np.ascontiguousarray(
                    np.asarray(key[sl], dtype=np.float32).transpose(0, 2, 1)
                ),
                "v": np.ascontiguousarray(value[sl], dtype=np.float32),
            }
        )
    res = run_bass_kernel_spmd(
        nc, in_maps, core_ids=list(range(N_CORES)), trace=_trace
    )
    out = np.concatenate(
        [
            res.results[c]["o"].transpose(0, 2, 1)
            / res.results[c]["l"].reshape(BPC, S)[:, :, None]
            for c in range(N_CORES)
        ],
        axis=0,
    )
    out = np.ascontiguousarray(out, dtype=np.float32)
    if _trace:
        return out, res
    return out



# revision 2
# speedup vs baseline: 1.3570x; 1.3570x over previous
"""Causal attention kernel for Trainium2 (Bass/Tile), 8-core SPMD.

Problem: B=16, S=2048, D=128 fp32 causal attention
    scores = Q @ K^T            (per batch)
    scores -= INF * triu(k=1)   (before scaling, as in reference)
    attn = softmax(scores / sqrt(D))
    out = attn @ V
Sharding: batch dim across 8 cores, 2 batches per core, no communication.

v2 (fp16 rework of the fp32r baseline):
  - All matmul operands fp16 (host casts).  Same 1 col/cycle PE stream
    rate as fp32r at >=256 cols, but no 4x penalty on <256-col matmuls,
    and FWL halves LDWEIGHTS (fp16 weights load 2/cycle).
  - Host packs K^T/Q^T/V-chunked into one contiguous [128, 6144] fp16
    tile per batch -> 2 dma_starts x 128 descriptors of 6KB (the fp32r
    baseline's V gather was 4096 x 512B descriptors per batch).
  - exp: ACT reads fp32 PSUM scores, writes fp16 P directly.
  - O^T ships fp16 (halves output DMA); l ships fp32; host divides.
  - Warmup runs first: memset dummy -> ACT table load -> 10x 512-col
    PE matmuls to lift HAM to K=8/8 while the first DMAs land.

Per-core dataflow per batch, per 512-wide q-block, per k-chunk group:
    S^T[k, q] = (K^T chunk).T @ Q^T slice      (contract d on partitions)
    diag chunks: causal mask written by a PE const matmul (u01.T @ NEG*I,
                 start=True), scores accumulate on top with start=False
    P^T = exp(S^T * 1/sqrt(D))                 (ACT, PSUM fp32 -> SBUF fp16)
    O^T[d, q] += V_chunk.T @ P^T chunk          (PSUM fp32 accumulate)
    l[q]      += allones.T @ P^T chunk          (rowsum, all rows equal)
Software pipelining: PV/rowsum of group g is emitted after S+exp of group
g+1; each q-block's evacuation is deferred past the next block's first
group, so the in-order PE queue never stalls on ACT or DVE chains.
"""

import os

os.environ.setdefault("MYCRO_LOCAL_CACHE", "1")

import math

import numpy as np

import concourse.bass as bass
import concourse.mybir as mybir
import concourse.tile as tile
from concourse import bacc
from concourse.bass_utils import run_bass_kernel_spmd
from concourse.masks import make_identity

F32 = mybir.dt.float32
F16 = mybir.dt.float16
EXPF = mybir.ActivationFunctionType.Exp

N_CORES = 8
B = 16
S = 2048
D = 128
BPC = B // N_CORES  # batches per core
SCALE = 1.0 / math.sqrt(float(D))
# Finite in fp16 (so 0*NEG never makes NaN in the mask matmul); after
# *SCALE it underflows exp to exactly 0.0, same as the reference's -1e10.
NEG = -57344.0
NQB = S // 512  # q blocks per batch
NCH = S // 128  # k chunks per batch

# packed qkv column map: piece0 = [kt 0:1024 | qt 0:1024 | vr 0:1024],
# piece1 same for cols 1024:2048.  All kernel slices stay within one
# 1024-col sub-block (q-blocks are 512-aligned, chunks 128-aligned).


def _kt_col(c):
    return c if c < 1024 else 3072 + (c - 1024)


def _qt_col(c):
    return 1024 + c if c < 1024 else 4096 + (c - 1024)


def _vr_col(c):
    return 2048 + c if c < 1024 else 5120 + (c - 1024)


def build():
    nc = bacc.Bacc("TRN2", target_bir_lowering=False, debug=False, num_devices=N_CORES)
    x_d = nc.dram_tensor("x", [BPC, 128, 6144], F16, kind="ExternalInput")
    o_d = nc.dram_tensor("o", [BPC, 128, S], F16, kind="ExternalOutput")
    l_d = nc.dram_tensor("l", [BPC, NQB, 1, 512], F32, kind="ExternalOutput")

    with tile.TileContext(nc) as tc:
        with (
            tc.tile_pool(name="const", bufs=1) as constp,
            tc.tile_pool(name="qkv", bufs=2) as qkvp,
            tc.tile_pool(name="pt", bufs=6) as ptp,
            tc.tile_pool(name="evac", bufs=2) as evacp,
            tc.tile_pool(name="stps", bufs=3, space="PSUM") as stps,
            tc.tile_pool(name="otps", bufs=1, space="PSUM") as otps,
            tc.tile_pool(name="lps", bufs=1, space="PSUM") as lps,
        ):
            # ---- warmup: ACT table load + PE HAM ramp, before anything ----
            dummy = constp.tile([128, 512], F16, name="dummy")
            nc.gpsimd.memset(dummy[:], 0.0)
            warm_exp = constp.tile([128, 1], F32, name="warm_exp")
            nc.scalar.activation(warm_exp[:], dummy[:, 0:1], EXPF, bias=0.0, scale=SCALE)
            warm_ps = stps.tile([128, 512], F32, name="warm_ps", tag="stps")
            for _ in range(10):
                nc.tensor.matmul(
                    warm_ps[:], dummy[:, 0:128], dummy[:], start=True, stop=True
                )

            # ---- mask consts (fp16) ----
            ident = constp.tile([128, 128], F32, name="ident")
            make_identity(nc, ident[:])
            u01_f = constp.tile([128, 128], F32, name="u01_f")
            nc.gpsimd.memset(u01_f[:], 1.0)
            # u01[p, c] = 1 iff c > p
            nc.gpsimd.affine_select(
                out=u01_f[:],
                in_=u01_f[:],
                compare_op=mybir.AluOpType.is_ge,
                fill=0.0,
                base=-1,
                pattern=[[1, 128]],
                channel_multiplier=-1,
            )
            u01_h = constp.tile([128, 128], F16, name="u01_h")
            idneg_h = constp.tile([128, 128], F16, name="idneg_h")
            idneg2_h = constp.tile([128, 384], F16, name="idneg2_h")
            ones_h = constp.tile([128, 128], F16, name="ones_h")
            with nc.allow_low_precision("fp16 mask/ones consts are exact"):
                nc.vector.tensor_copy(u01_h[:], u01_f[:])
                nc.vector.tensor_scalar_mul(idneg_h[:], ident[:], NEG)
                nc.gpsimd.memset(idneg2_h[:], 0.0)
                nc.vector.tensor_scalar_mul(idneg2_h[:, 0:128], ident[:], NEG)
                nc.vector.tensor_scalar_mul(idneg2_h[:, 256:384], ident[:], NEG)
                nc.gpsimd.memset(ones_h[:], 1.0)

            # software-pipeline state (see module docstring)
            pending_pv = [None]
            pending_evac = [None]

            def flush_pv():
                if pending_pv[0] is not None:
                    pending_pv[0]()
                    pending_pv[0] = None

            def flush_evac():
                if pending_evac[0] is not None:
                    pending_evac[0]()
                    pending_evac[0] = None

            for b in range(BPC):
                qkv = qkvp.tile([128, 6144], F16, name="qkv")
                nc.sync.dma_start(qkv[:, 0:3072], x_d[b, :, 0:3072])
                nc.sync.dma_start(qkv[:, 3072:6144], x_d[b, :, 3072:6144])

                def kt_ap(j, qkv=qkv):
                    c = _kt_col(j * 128)
                    return qkv[:, c : c + 128]

                def qt_ap(c0, w, qkv=qkv):
                    c = _qt_col(c0)
                    return qkv[:, c : c + w]

                def vr_ap(j, qkv=qkv):
                    c = _vr_col(j * 128)
                    return qkv[:, c : c + 128]

                for qb in range(NQB):
                    n_full = 4 * qb
                    n_ch = n_full + 4
                    q0 = qb * 512

                    # (chunks, extent, is_diag); chunk = (j, qoff, width, col)
                    groups = []
                    jf = 0
                    while jf < n_full:
                        g = min(2, n_full - jf)
                        groups.append(
                            (
                                [(jf + c, 0, 512, c * 512) for c in range(g)],
                                g * 512,
                                False,
                            )
                        )
                        jf += g
                    groups.append(
                        ([(n_full, 0, 512, 0), (n_full + 1, 128, 384, 512)], 896, True)
                    )
                    groups.append(
                        (
                            [(n_full + 2, 256, 256, 0), (n_full + 3, 384, 128, 256)],
                            384,
                            True,
                        )
                    )

                    ot = otps.tile([128, 512], F32, name="ot")
                    lp = lps.tile([128, 512], F32, name="lp", tag="lp")

                    for gi, (chunks, extent, is_diag) in enumerate(groups):
                        st = stps.tile([128, 1024], F32, name="st", tag="stps")
                        if (
                            is_diag
                            and chunks[0][3] == 0
                            and len(chunks) == 2
                            and chunks[1][3] == 256
                        ):
                            # diag B: one mask matmul covers both chunks
                            nc.tensor.matmul(
                                st[:, 0:384],
                                u01_h[:],
                                idneg2_h[:],
                                start=True,
                                stop=False,
                            )
                            premasked = True
                        else:
                            premasked = False
                        for (j, qoff, width, col) in chunks:
                            if is_diag and not premasked:
                                nc.tensor.matmul(
                                    st[:, col : col + 128],
                                    u01_h[:],
                                    idneg_h[:],
                                    start=True,
                                    stop=False,
                                )
                            nc.tensor.matmul(
                                st[:, col : col + width],
                                kt_ap(j),
                                qt_ap(q0 + qoff, width),
                                start=not is_diag,
                                stop=True,
                            )
                        pt = ptp.tile([128, 1024], F16, name="pt", tag="pt")
                        with nc.allow_low_precision("fp16 P within tolerance"):
                            nc.scalar.activation(
                                pt[:, 0:extent],
                                st[:, 0:extent],
                                EXPF,
                                bias=0.0,
                                scale=SCALE,
                            )
                        flush_pv()
                        if gi == 1:
                            flush_evac()

                        def pv(
                            chunks=chunks,
                            ot=ot,
                            lp=lp,
                            pt=pt,
                            n_ch=n_ch,
                            is_last=(gi == len(groups) - 1),
                            b=b,
                            q0=q0,
                            qb=qb,
                            vr_ap=vr_ap,
                        ):
                            for (j, qoff, width, col) in chunks:
                                nc.tensor.matmul(
                                    ot[:, qoff : qoff + width],
                                    vr_ap(j),
                                    pt[:, col : col + width],
                                    start=(j == 0),
                                    stop=(j == n_ch - 1),
                                )
                                nc.tensor.matmul(
                                    lp[:, qoff : qoff + width],
                                    ones_h[:],
                                    pt[:, col : col + width],
                                    start=(j == 0),
                                    stop=(j == n_ch - 1),
                                )
                            if not is_last:
                                return
                            # evacuation: unnormalized O^T (fp16) + rowsum
                            # row; the host divides in fp64.
                            ots = evacp.tile([128, 512], F16, name="ots")
                            lsb = evacp.tile([1, 512], F32, name="lsb")
                            with nc.allow_low_precision("fp16 O^T ship"):
                                nc.vector.tensor_copy(ots[:], ot[:])
                            nc.vector.tensor_copy(lsb[:], lp[0:1, :])

                            def evac(b=b, q0=q0, qb=qb, ots=ots, lsb=lsb):
                                nc.sync.dma_start(o_d[b, :, q0 : q0 + 512], ots[:])
                                nc.sync.dma_start(l_d[b, qb], lsb[:])

                            pending_evac[0] = evac

                        pending_pv[0] = pv

            flush_pv()
            flush_evac()
    nc.compile()
    return nc


_NC_CACHE = None


def _get_nc():
    global _NC_CACHE
    if _NC_CACHE is None:
        _NC_CACHE = build()
    return _NC_CACHE


def kernel(query, key, value, _trace=False):
    nc = _get_nc()
    in_maps = []
    for c in range(N_CORES):
        sl = slice(c * BPC, (c + 1) * BPC)
        q = np.asarray(query[sl], dtype=np.float32).astype(np.float16)
        k = np.asarray(key[sl], dtype=np.float32).astype(np.float16)
        v = np.asarray(value[sl], dtype=np.float32).astype(np.float16)
        x = np.empty((BPC, 128, 6144), np.float16)
        for b in range(BPC):
            ktp = k[b].T  # [128 d, 2048 s]
            qtp = q[b].T
            vrp = np.ascontiguousarray(
                v[b].reshape(16, 128, 128).transpose(1, 0, 2)
            ).reshape(128, 2048)
            x[b, :, 0:1024] = ktp[:, 0:1024]
            x[b, :, 1024:2048] = qtp[:, 0:1024]
            x[b, :, 2048:3072] = vrp[:, 0:1024]
            x[b, :, 3072:4096] = ktp[:, 1024:2048]
            x[b, :, 4096:5120] = qtp[:, 1024:2048]
            x[b, :, 5120:6144] = vrp[:, 1024:2048]
        in_maps.append({"x": x})
    res = run_bass_kernel_spmd(
        nc, in_maps, core_ids=list(range(N_CORES)), trace=_trace
    )
    outs = []
    for c in range(N_CORES):
        o = res.results[c]["o"].astype(np.float32)  # [BPC, 128, 2048]
        l = res.results[c]["l"].reshape(BPC, S).astype(np.float32)
        outs.append(o.transpose(0, 2, 1) / l[:, :, None])
    out = np.ascontiguousarray(np.concatenate(outs, axis=0), dtype=np.float32)
    if _trace:
        return out, res
    return out


# revision 4
# speedup vs baseline: 1.5043x; 1.1085x over previous
"""Causal attention kernel for Trainium2 (Bass/Tile), 8-core SPMD.

Problem: B=16, S=2048, D=128 fp32 causal attention
    scores = Q @ K^T; scores -= INF*triu(k=1); attn = softmax(scores/sqrt(D));
    out = attn @ V.   Batch dim sharded across 8 cores, 2 batches per core.

v3 (on top of the fp16 rework):
  - exp computes P' = exp(s/sqrt(D) - 2): softmax is shift-invariant (host
    divide cancels the e^-2), and P' <= e^4 keeps fp8e4m3 casts safe.
  - Rowsum l for the full (non-diagonal) chunk groups runs as ONE fp8
    DoubleRow matmul per 2-chunk group (contract 256 rows at 2 fp8/cycle):
    half the PE cycles and a quarter of the instructions of the fp16
    per-chunk version.  All-ones weights make the DR k-interleave order
    irrelevant.  pt -> pt8 casts run on the otherwise-idle DVE.
    Diagonal groups keep exact fp16 rowsums (rows with few attended keys
    can't absorb fp8 numerator/denominator mismatch).
  - Causal masking: instead of PE premask matmuls writing NEG into PSUM,
    GPSIMD affine_select zeroes the mask triangle of P directly in SBUF
    after the exp (keep q >= k), freeing PE cycles and all mask consts.
  - Warmup: 8x 512-col matmuls on a memset dummy lift HAM to K=8/8 while
    the first DMAs land; a dummy exp pre-loads the ACT exp table.

Dataflow per batch / 512-wide q-block / k-chunk group (as v2):
    S^T[k,q] = (K^T chunk).T @ Q^T slice -> fp32 PSUM
    P^T = exp(S^T/sqrt(D) - 2)           -> fp16 SBUF (ACT)
    O^T[d,q] += V_chunk.T @ P^T chunk    -> fp32 PSUM (fp16 PE)
    l[q]     += rowsum(P^T)              -> fp32 PSUM (fp8-DR / fp16 PE)
    host: out = (O^T / l).T  in fp64.
Software pipelining: PV/rowsum of group g is emitted after S+exp of group
g+1; each q-block's evacuation is deferred past the next block's first
group, so the in-order PE queue never stalls on ACT or DVE chains.
"""

import os

os.environ.setdefault("MYCRO_LOCAL_CACHE", "1")

import math

import numpy as np

import concourse.bass as bass
import concourse.mybir as mybir
import concourse.tile as tile
from concourse import bacc
from concourse.bass_utils import run_bass_kernel_spmd

F32 = mybir.dt.float32
F16 = mybir.dt.float16
F8 = mybir.dt.float8e4
EXPF = mybir.ActivationFunctionType.Exp
DR = mybir.MatmulPerfMode.DoubleRow

N_CORES = 8
B = 16
S = 2048
D = 128
BPC = B // N_CORES
SCALE = 1.0 / math.sqrt(float(D))
SHIFT = -2.0  # exp bias; cancels in O/l, keeps P' <= e^4 fp8-safe
NQB = S // 512
NCH = S // 128

USE_DR_ROWSUM = True
USE_GPSIMD_MASK = True

# packed qkv column map: piece0 = [kt 0:1024 | qt 0:1024 | vr 0:1024],
# piece1 the same for source cols 1024:2048.


def _kt_col(c):
    return c if c < 1024 else 3072 + (c - 1024)


def _qt_col(c):
    return 1024 + c if c < 1024 else 4096 + (c - 1024)


def _vr_col(c):
    return 2048 + c if c < 1024 else 5120 + (c - 1024)


def build():
    nc = bacc.Bacc("TRN2", target_bir_lowering=False, debug=False, num_devices=N_CORES)
    x_d = nc.dram_tensor("x", [BPC, 128, 6144], F16, kind="ExternalInput")
    o_d = nc.dram_tensor("o", [BPC, 128, S], F16, kind="ExternalOutput")
    l_d = nc.dram_tensor("l", [BPC, NQB, 1, 512], F32, kind="ExternalOutput")

    with tile.TileContext(nc) as tc:
        with (
            tc.tile_pool(name="const", bufs=1) as constp,
            tc.tile_pool(name="qkv", bufs=2) as qkvp,
            tc.tile_pool(name="pt", bufs=6) as ptp,
            tc.tile_pool(name="pt8", bufs=4) as pt8p,
            tc.tile_pool(name="evac", bufs=2) as evacp,
            tc.tile_pool(name="stps", bufs=3, space="PSUM") as stps,
            tc.tile_pool(name="otps", bufs=1, space="PSUM") as otps,
            tc.tile_pool(name="lps", bufs=1, space="PSUM") as lps,
        ):
            # ---- warmup: ACT table load + PE HAM ramp, before anything ----
            dummy = constp.tile([128, 512], F16, name="dummy")
            nc.gpsimd.memset(dummy[:], 0.0)
            shiftb = constp.tile([128, 1], F32, name="shiftb")
            nc.gpsimd.memset(shiftb[:], SHIFT)
            warm_exp = constp.tile([128, 1], F32, name="warm_exp")
            nc.scalar.activation(
                warm_exp[:], dummy[:, 0:1], EXPF, bias=shiftb[:], scale=SCALE
            )
            warm_ps = stps.tile([128, 512], F32, name="warm_ps", tag="stps")
            for _ in range(8):
                nc.tensor.matmul(
                    warm_ps[:], dummy[:, 0:128], dummy[:], start=True, stop=True
                )

            # ---- consts ----
            ones_h = constp.tile([128, 128], F16, name="ones_h")
            nc.gpsimd.memset(ones_h[:], 1.0)
            if USE_DR_ROWSUM:
                ones8 = constp.tile([128, 256], F8, name="ones8")
                nc.gpsimd.memset(ones8[:], 1.0)
            if not USE_GPSIMD_MASK:
                from concourse.masks import make_identity

                NEGC = -57344.0
                ident = constp.tile([128, 128], F32, name="ident")
                make_identity(nc, ident[:])
                u01_f = constp.tile([128, 128], F32, name="u01_f")
                nc.gpsimd.memset(u01_f[:], 1.0)
                nc.gpsimd.affine_select(
                    out=u01_f[:],
                    in_=u01_f[:],
                    compare_op=mybir.AluOpType.is_ge,
                    fill=0.0,
                    base=-1,
                    pattern=[[1, 128]],
                    channel_multiplier=-1,
                )
                u01_h = constp.tile([128, 128], F16, name="u01_h")
                idneg_h = constp.tile([128, 128], F16, name="idneg_h")
                idneg2_h = constp.tile([128, 384], F16, name="idneg2_h")
                with nc.allow_low_precision("fp16 mask consts are exact"):
                    nc.vector.tensor_copy(u01_h[:], u01_f[:])
                    nc.vector.tensor_scalar_mul(idneg_h[:], ident[:], NEGC)
                    nc.gpsimd.memset(idneg2_h[:], 0.0)
                    nc.vector.tensor_scalar_mul(idneg2_h[:, 0:128], ident[:], NEGC)
                    nc.vector.tensor_scalar_mul(idneg2_h[:, 256:384], ident[:], NEGC)

            pending_pv = [None]
            pending_evac = [None]

            def flush_pv():
                if pending_pv[0] is not None:
                    pending_pv[0]()
                    pending_pv[0] = None

            def flush_evac():
                if pending_evac[0] is not None:
                    pending_evac[0]()
                    pending_evac[0] = None

            for b in range(BPC):
                qkv = qkvp.tile([128, 6144], F16, name="qkv")
                nc.sync.dma_start(qkv[:, 0:3072], x_d[b, :, 0:3072])
                nc.sync.dma_start(qkv[:, 3072:6144], x_d[b, :, 3072:6144])

                def kt_ap(j, qkv=qkv):
                    c = _kt_col(j * 128)
                    return qkv[:, c : c + 128]

                def qt_ap(c0, w, qkv=qkv):
                    c = _qt_col(c0)
                    return qkv[:, c : c + w]

                def vr_ap(j, qkv=qkv):
                    c = _vr_col(j * 128)
                    return qkv[:, c : c + 128]

                for qb in range(NQB):
                    n_full = 4 * qb
                    n_ch = n_full + 4
                    q0 = qb * 512

                    # (chunks, extent, is_diag); chunk = (j, qoff, width, col)
                    groups = []
                    jf = 0
                    while jf < n_full:
                        g = min(2, n_full - jf)
                        groups.append(
                            (
                                [(jf + c, 0, 512, c * 512) for c in range(g)],
                                g * 512,
                                False,
                            )
                        )
                        jf += g
                    groups.append(
                        ([(n_full, 0, 512, 0), (n_full + 1, 128, 384, 512)], 896, True)
                    )
                    groups.append(
                        (
                            [(n_full + 2, 256, 256, 0), (n_full + 3, 384, 128, 256)],
                            384,
                            True,
                        )
                    )

                    ot = otps.tile([128, 512], F32, name="ot")
                    lp = lps.tile([128, 512], F32, name="lp", tag="lp")

                    for gi, (chunks, extent, is_diag) in enumerate(groups):
                        st = stps.tile([128, 1024], F32, name="st", tag="stps")
                        if not USE_GPSIMD_MASK and is_diag:
                            if chunks[0][3] == 0 and chunks[1][3] == 256:
                                nc.tensor.matmul(
                                    st[:, 0:384],
                                    u01_h[:],
                                    idneg2_h[:],
                                    start=True,
                                    stop=False,
                                )
                                premasked = True
                            else:
                                premasked = False
                        else:
                            premasked = True  # no PE premask needed
                        for (j, qoff, width, col) in chunks:
                            qk_start = True
                            if not USE_GPSIMD_MASK and is_diag:
                                if not premasked:
                                    nc.tensor.matmul(
                                        st[:, col : col + 128],
                                        u01_h[:],
                                        idneg_h[:],
                                        start=True,
                                        stop=False,
                                    )
                                qk_start = False
                            nc.tensor.matmul(
                                st[:, col : col + width],
                                kt_ap(j),
                                qt_ap(q0 + qoff, width),
                                start=qk_start,
                                stop=True,
                            )
                        pt = ptp.tile([128, 1024], F16, name="pt", tag="pt")
                        with nc.allow_low_precision("fp16 P within tolerance"):
                            nc.scalar.activation(
                                pt[:, 0:extent],
                                st[:, 0:extent],
                                EXPF,
                                bias=shiftb[:],
                                scale=SCALE,
                            )
                        if USE_GPSIMD_MASK and is_diag:
                            # zero P where q < k (keep local col >= partition)
                            for (j, qoff, width, col) in chunks:
                                nc.gpsimd.affine_select(
                                    out=pt[:, col : col + 128],
                                    in_=pt[:, col : col + 128],
                                    compare_op=mybir.AluOpType.is_ge,
                                    fill=0.0,
                                    base=0,
                                    pattern=[[1, 128]],
                                    channel_multiplier=-1,
                                )
                        pt8 = None
                        if USE_DR_ROWSUM and not is_diag and len(chunks) == 2:
                            pt8 = pt8p.tile([128, 1024], F8, name="pt8", tag="pt8")
                            with nc.allow_low_precision("fp8 rowsum only"):
                                nc.vector.tensor_copy(pt8[:], pt[:])
                        flush_pv()
                        if gi == 1:
                            flush_evac()

                        def pv(
                            chunks=chunks,
                            ot=ot,
                            lp=lp,
                            pt=pt,
                            pt8=pt8,
                            n_ch=n_ch,
                            is_last=(gi == len(groups) - 1),
                            b=b,
                            q0=q0,
                            qb=qb,
                            vr_ap=vr_ap,
                        ):
                            for (j, qoff, width, col) in chunks:
                                nc.tensor.matmul(
                                    ot[:, qoff : qoff + width],
                                    vr_ap(j),
                                    pt[:, col : col + width],
                                    start=(j == 0),
                                    stop=(j == n_ch - 1),
                                )
                            if pt8 is not None:
                                j0 = chunks[0][0]
                                nc.tensor.matmul(
                                    lp[:],
                                    ones8[:].rearrange("p (i m) -> p i m", i=2),
                                    pt8[:].rearrange("p (i n) -> p i n", i=2),
                                    start=(j0 == 0),
                                    stop=(j0 + 1 == n_ch - 1),
                                    perf_mode=DR,
                                )
                            else:
                                for (j, qoff, width, col) in chunks:
                                    nc.tensor.matmul(
                                        lp[:, qoff : qoff + width],
                                        ones_h[:],
                                        pt[:, col : col + width],
                                        start=(j == 0),
                                        stop=(j == n_ch - 1),
                                    )
                            if not is_last:
                                return
                            ots = evacp.tile([128, 512], F16, name="ots")
                            lsb = evacp.tile([1, 512], F32, name="lsb")
                            with nc.allow_low_precision("fp16 O^T ship"):
                                nc.vector.tensor_copy(ots[:], ot[:])
                            nc.vector.tensor_copy(lsb[:], lp[0:1, :])

                            def evac(b=b, q0=q0, qb=qb, ots=ots, lsb=lsb):
                                nc.sync.dma_start(o_d[b, :, q0 : q0 + 512], ots[:])
                                nc.sync.dma_start(l_d[b, qb], lsb[:])

                            if b == BPC - 1 and qb == NQB - 1:
                                evac()
                            else:
                                pending_evac[0] = evac

                        pending_pv[0] = pv

            flush_pv()
            flush_evac()
    nc.compile()
    return nc


_NC_CACHE = None


def _get_nc():
    global _NC_CACHE
    if _NC_CACHE is None:
        _NC_CACHE = build()
    return _NC_CACHE


def kernel(query, key, value, _trace=False):
    nc = _get_nc()
    in_maps = []
    for c in range(N_CORES):
        sl = slice(c * BPC, (c + 1) * BPC)
        q = np.asarray(query[sl], dtype=np.float32).astype(np.float16)
        k = np.asarray(key[sl], dtype=np.float32).astype(np.float16)
        v = np.asarray(value[sl], dtype=np.float32).astype(np.float16)
        x = np.empty((BPC, 128, 6144), np.float16)
        for b in range(BPC):
            ktp = k[b].T
            qtp = q[b].T
            vrp = np.ascontiguousarray(
                v[b].reshape(16, 128, 128).transpose(1, 0, 2)
            ).reshape(128, 2048)
            x[b, :, 0:1024] = ktp[:, 0:1024]
            x[b, :, 1024:2048] = qtp[:, 0:1024]
            x[b, :, 2048:3072] = vrp[:, 0:1024]
            x[b, :, 3072:4096] = ktp[:, 1024:2048]
            x[b, :, 4096:5120] = qtp[:, 1024:2048]
            x[b, :, 5120:6144] = vrp[:, 1024:2048]
        in_maps.append({"x": x})
    res = run_bass_kernel_spmd(
        nc, in_maps, core_ids=list(range(N_CORES)), trace=_trace
    )
    outs = []
    for c in range(N_CORES):
        o = res.results[c]["o"].astype(np.float32)
        l = res.results[c]["l"].reshape(BPC, S).astype(np.float32)
        outs.append(o.transpose(0, 2, 1) / l[:, :, None])
    out = np.ascontiguousarray(np.concatenate(outs, axis=0), dtype=np.float32)
    if _trace:
        return out, res
    return out
